# revision 1
# baseline (speedup 1.0000x reference)
"""MobileViTV2 block kernel for 8 TRN2 NeuronCores (data-parallel over batch).

Layout: d-major everywhere — features on SBUF partitions, tokens on the free
axis, token order n = t*1024 + h*32 + w (natural). Patch id of a token is
(h&1, w&1), recoverable from free-index bits, so attention runs on natural
order with strided sub-APs and nothing is ever transposed or scattered.

Per core (one batch element):
  P1: dwconv3x3x3 (DVE fused mul-add taps over a zero-padded slice window)
      + SiLU + pw1 matmul -> z0 chunks; fused block-0 "pass A": LN1 stats via
      ones-matmul, normalize, qkv matmul, exp(q) (no max-sub; LN-bounded),
      cv += cs*k via tensor_tensor_reduce, v=relu(v+b) -> DRAM.
  P2: block-0 "pass B" (v*cv -> wo -> +z residual, LN2+FFN residual) fused
      with block-1 pass A.
  P3: block-1 pass B fused with pw2 -> out.
GEMMs run as float32r (full-rate fp32). All host-side prep (LN-gain folding
into the next GEMM's weights, qkv split) happens in numpy inside kernel().
"""

import sys

sys.path.insert(0, "/opt/trn_rl_repo")
import os
import numpy as np
from contextlib import ExitStack

import concourse.bass as bass
import concourse.mybir as mybir
import concourse.tile as tile
from concourse import bacc
from concourse.bass_utils import run_bass_kernel_spmd

F32 = mybir.dt.float32
F32R = mybir.dt.float32r
AF = mybir.ActivationFunctionType
OP = mybir.AluOpType

B, C, T, H, W = 8, 256, 16, 32, 32
D, OUTC, NBLK, FF = 384, 256, 2, 768
NTOK = T * H * W  # 16384 tokens per batch element
CH = 512  # tokens per chunk
NCH = NTOK // CH  # 32
PW = 34  # padded spatial row
PSL = PW * PW  # padded slice 1156
EPS = 1e-5

# build stage knob for incremental bring-up (3 = full kernel)
STAGE = int(os.environ.get("KERNEL_STAGE", "3"))
# CoreSim doesn't implement Silu/Exp etc.; swap to sim-supported funcs when
# hunting memory/sync bugs (numerics intentionally wrong then).
SIM_SAFE = bool(int(os.environ.get("KERNEL_SIM_SAFE", "0")))
SKIP = set(os.environ.get("KERNEL_SKIP", "").split(","))


def _w_tiles(nc, wpool, name, dram, kdim, mdim, as_f32r=True):
    """Load a [K, M] DRAM weight as ceil(K/128) SBUF lhsT tiles."""
    tiles = []
    for ki in range((kdim + 127) // 128):
        kk = min(128, kdim - ki * 128)
        t = wpool.tile([128, mdim], F32, tag=f"{name}{ki}")
        dst = t[:kk, :]
        if as_f32r:
            dst = dst.bitcast(F32R)
        nc.sync.dma_start(out=dst, in_=dram[ki * 128 : ki * 128 + kk, :].bitcast(F32R) if as_f32r else dram[ki * 128 : ki * 128 + kk, :])
        tiles.append(t)
    return tiles


def _bias_tile(nc, wpool, name, dram, n):
    """Load a [n,1] DRAM bias as a [128, ceil(n/128)] SBUF tile (col per ktile)."""
    nt = (n + 127) // 128
    t = wpool.tile([128, nt], F32, tag=name)
    for ki in range(nt):
        kk = min(128, n - ki * 128)
        nc.sync.dma_start(out=t[:kk, ki : ki + 1], in_=dram[ki * 128 : ki * 128 + kk, :])
    return t


def patch_view(ap):
    """[p, 512] -> [p, 8, 2, 16, 2]; dims 2/4 are the (ph, pw) patch bits."""
    return ap.rearrange("p (a b c d) -> p a b c d", a=8, b=2, c=16, d=2)


def build():
    nc = bacc.Bacc("TRN2", target_bir_lowering=False, debug=False, num_devices=8)

    x_in = nc.dram_tensor("x", [C, T, H, W], F32, kind="ExternalInput").ap()
    dwW = nc.dram_tensor("dwW", [C, 27], F32, kind="ExternalInput").ap()
    dwB = nc.dram_tensor("dwB", [C, 1], F32, kind="ExternalInput").ap()
    pw1W = nc.dram_tensor("pw1W", [C, D], F32, kind="ExternalInput").ap()
    pw1B = nc.dram_tensor("pw1B", [D, 1], F32, kind="ExternalInput").ap()
    pw2W = nc.dram_tensor("pw2W", [D, OUTC], F32, kind="ExternalInput").ap()
    pw2B = nc.dram_tensor("pw2B", [OUTC, 1], F32, kind="ExternalInput").ap()
    blk = []
    for i in range(NBLK):
        blk.append(
            dict(
                qkvW=nc.dram_tensor(f"qkvW{i}", [D, 1 + 2 * D], F32, kind="ExternalInput").ap(),
                qB=nc.dram_tensor(f"qB{i}", [1, 1], F32, kind="ExternalInput").ap(),
                kB=nc.dram_tensor(f"kB{i}", [D, 1], F32, kind="ExternalInput").ap(),
                vB=nc.dram_tensor(f"vB{i}", [D, 1], F32, kind="ExternalInput").ap(),
                woW=nc.dram_tensor(f"woW{i}", [D, D], F32, kind="ExternalInput").ap(),
                woB=nc.dram_tensor(f"woB{i}", [D, 1], F32, kind="ExternalInput").ap(),
                ff1W=nc.dram_tensor(f"ff1W{i}", [D, FF], F32, kind="ExternalInput").ap(),
                ff1B=nc.dram_tensor(f"ff1B{i}", [FF, 1], F32, kind="ExternalInput").ap(),
                ff2W=nc.dram_tensor(f"ff2W{i}", [FF, D], F32, kind="ExternalInput").ap(),
                ff2B=nc.dram_tensor(f"ff2B{i}", [D, 1], F32, kind="ExternalInput").ap(),
            )
        )
    out = nc.dram_tensor("out", [OUTC, NTOK], F32, kind="ExternalOutput").ap()
    z0 = nc.dram_tensor("z0", [D, NTOK], F32, kind="ExternalOutput").ap()
    z1 = nc.dram_tensor("z1", [D, NTOK], F32, kind="ExternalOutput").ap()
    v0 = nc.dram_tensor("v0", [D, NTOK], F32).ap()
    v1 = nc.dram_tensor("v1", [D, NTOK], F32).ap()
    csd = nc.dram_tensor("csd", [NCH, CH], F32).ap()
    zsd = nc.dram_tensor("zsd", [NBLK, 4], F32).ap()

    with ExitStack() as ctx:
        tc = ctx.enter_context(tile.TileContext(nc))
        wpool = ctx.enter_context(tc.tile_pool(name="w", bufs=1))
        sp = ctx.enter_context(tc.tile_pool(name="s", bufs=2))
        pp = ctx.enter_context(tc.tile_pool(name="ps", bufs=8, space="PSUM"))
        # phase-local pools are opened/closed inside build body
        cvp = ctx.enter_context(tc.tile_pool(name="cv", bufs=1))

        # ---- weights ----
        dw_t = wpool.tile([128, 2, 27], F32, tag="dwW")  # [part, chtile, tap]
        for cti in range(2):
            nc.sync.dma_start(out=dw_t[:, cti, :], in_=dwW[cti * 128 : (cti + 1) * 128, :])
        dwb_t = _bias_tile(nc, wpool, "dwB", dwB, C)
        pw1_t = _w_tiles(nc, wpool, "pw1W", pw1W, C, D)
        pw1b_t = _bias_tile(nc, wpool, "pw1B", pw1B, D)
        pw2_t = _w_tiles(nc, wpool, "pw2W", pw2W, D, OUTC)
        pw2b_t = _bias_tile(nc, wpool, "pw2B", pw2B, OUTC)
        bw = []
        for i in range(NBLK):
            bw.append(
                dict(
                    qkv=_w_tiles(nc, wpool, f"qkvW{i}_", blk[i]["qkvW"], D, 1 + 2 * D),
                    qB=_bias_tile(nc, wpool, f"qB{i}", blk[i]["qB"], 1),
                    kB=_bias_tile(nc, wpool, f"kB{i}", blk[i]["kB"], D),
                    vB=_bias_tile(nc, wpool, f"vB{i}", blk[i]["vB"], D),
                    wo=_w_tiles(nc, wpool, f"woW{i}_", blk[i]["woW"], D, D),
                    woB=_bias_tile(nc, wpool, f"woB{i}", blk[i]["woB"], D),
                    ff1=_w_tiles(nc, wpool, f"ff1W{i}_", blk[i]["ff1W"], D, FF),
                    ff1B=_bias_tile(nc, wpool, f"ff1B{i}", blk[i]["ff1B"], FF),
                    ff2=_w_tiles(nc, wpool, f"ff2W{i}_", blk[i]["ff2W"], FF, D),
                    ff2B=_bias_tile(nc, wpool, f"ff2B{i}", blk[i]["ff2B"], D),
                )
            )
        ones_f = wpool.tile([128, 128], F32, tag="onesf")
        nc.vector.memset(ones_f[:], 1.0)
        ones_t = wpool.tile([128, 128], F32, tag="ones")
        nc.scalar.copy(ones_t[:].bitcast(F32R), ones_f[:])
        eps_t = wpool.tile([128, 1], F32, tag="eps")
        nc.vector.memset(eps_t[:], EPS)

        # per-block attention state: cv ping-pong + Z partials + final cv
        att = []
        for i in range(NBLK):
            att.append(
                dict(
                    cvacc=cvp.tile([128, 3, 4], F32, tag=f"cvacc{i}", name=f"cvacc{i}"),  # [p, dtile, patch]
                    Zp=cvp.tile([1, 4, NCH], F32, tag=f"Zp{i}", name=f"Zp{i}"),
                    cvf=cvp.tile([128, 3, 4], F32, tag=f"cvf{i}", name=f"cvf{i}"),  # final cv * 1/Z
                )
            )
            nc.vector.memset(att[i]["cvacc"][:], 0.0)
            nc.vector.memset(att[i]["Zp"][:], 1.0)

        def ln_then_qkv_attn(bi, zt, chunk):
            """Pass A for block bi on an SBUF z chunk [128, 3*512] (f32r-written).

            LN1 stats -> normalize in place -> qkv -> exp(q)/Z partial,
            cv TTR accumulate, v -> relu -> DRAM (v0/v1).
            """
            a = att[bi]
            wts = bw[bi]
            vdst = v0 if bi == 0 else v1
            # sums via ones-matmul (replicated over partitions)
            if "sums" not in SKIP:
                ps_s = pp.tile([128, CH], F32, tag="ps")
                ps_q = pp.tile([128, CH], F32, tag="ps")
                for d in range(3):
                    sq = sp.tile([128, CH], F32, tag="sq", name="sq")
                    nc.scalar.activation(out=sq[:].bitcast(F32R), in_=zt[:, d * CH : (d + 1) * CH], func=AF.Square)
                    nc.tensor.matmul(ps_s[:], ones_t[:].bitcast(F32R), zt[:, d * CH : (d + 1) * CH].bitcast(F32R), start=(d == 0), stop=(d == 2))
                    nc.tensor.matmul(ps_q[:], ones_t[:].bitcast(F32R), sq[:].bitcast(F32R), start=(d == 0), stop=(d == 2))
                # stats: M = s/384; var = q/384 - M^2; R = 1/sqrt(var+eps)
                Mt = sp.tile([128, CH], F32, tag="Mt")
                Rt = sp.tile([128, CH], F32, tag="Rt")
                tmp = sp.tile([128, CH], F32, tag="tmp")
                nc.scalar.activation(out=Mt[:], in_=ps_s[:], func=AF.Copy, scale=1.0 / D)
                nc.vector.tensor_mul(tmp[:], Mt[:], Mt[:])
                nc.vector.scalar_tensor_tensor(out=tmp[:], in0=ps_q[:], scalar=1.0 / D, in1=tmp[:], op0=OP.mult, op1=OP.subtract)
                nc.scalar.activation(out=tmp[:], in_=tmp[:], func=AF.Sqrt, bias=eps_t[:])
                nc.vector.reciprocal(Rt[:], tmp[:])
                # normalize in place: zn = (z - M) * R  (M/R broadcast across dtiles)
                Mb = bass.AP(tensor=Mt[:].tensor, offset=Mt[:].offset, ap=[Mt[:].ap[0], [0, 3], [1, CH]])
                Rb = bass.AP(tensor=Rt[:].tensor, offset=Rt[:].offset, ap=[Rt[:].ap[0], [0, 3], [1, CH]])
                z3 = zt[:].rearrange("p (d n) -> p d n", d=3)
                nc.vector.tensor_sub(z3.bitcast(F32R), z3, Mb)
                nc.vector.tensor_mul(z3.bitcast(F32R), z3, Rb)
            # qkv
            ps_qq = pp.tile([128, CH], F32, tag="ps")
            if "q" not in SKIP:
                for k in range(3):
                    nc.tensor.matmul(ps_qq[0:1, :], wts["qkv"][k][:, 0:1].bitcast(F32R), zt[:, k * CH : (k + 1) * CH].bitcast(F32R), start=(k == 0), stop=(k == 2))
            # softmax numerator: cs = exp(q + qB) per patch; Z partial via accum
            cs = sp.tile([1, CH], F32, tag="cs")
            if "exp" not in SKIP and "q" not in SKIP:
                qv = patch_view(ps_qq[0:1, :])
                cv_ = patch_view(cs[:])
                for ph in range(2):
                    for pw_ in range(2):
                        nc.scalar.activation(
                            out=cv_[:, :, ph, :, pw_],
                            in_=qv[:, :, ph, :, pw_],
                            func=AF.Exp,
                            bias=wts["qB"][0:1, 0:1],
                            accum_out=a["Zp"][0:1, 2 * ph + pw_, chunk : chunk + 1],
                        )
            else:
                nc.vector.memset(cs[:], 0.5)
            # broadcast cs across 128 partitions via DRAM bounce
            nc.sync.dma_start(out=csd[chunk : chunk + 1, :], in_=cs[:])
            csb = sp.tile([128, CH], F32, tag="csb")
            csrow = csd[chunk, :]
            nc.sync.dma_start(out=csb[:], in_=bass.AP(tensor=csrow.tensor, offset=csrow.offset, ap=[[0, 128], [1, CH]]))
            cvch = sp.tile([128, 3, 4], F32, tag="cvch")
            for m in range(3):
                ps_k = pp.tile([128, CH], F32, tag="ps")
                for k in range(3):
                    nc.tensor.matmul(ps_k[:], wts["qkv"][k][:, 1 + m * 128 : 1 + (m + 1) * 128].bitcast(F32R), zt[:, k * CH : (k + 1) * CH].bitcast(F32R), start=(k == 0), stop=(k == 2))
                kt = sp.tile([128, CH], F32, tag="kt", name="kt")
                nc.scalar.activation(out=kt[:], in_=ps_k[:], func=AF.Identity, bias=wts["kB"][:, m : m + 1])
                junk = sp.tile([128, CH], F32, tag="junk", name="junk")
                nc.vector.tensor_mul(junk[:], kt[:], csb[:])
                jv = patch_view(junk[:])
                for ph in range(2 if "ttr" not in SKIP else 0):
                    for pw_ in range(2):
                        p = 2 * ph + pw_
                        nc.scalar.activation(
                            out=jv[:, :, ph, :, pw_],
                            in_=jv[:, :, ph, :, pw_],
                            func=AF.Copy,
                            accum_out=cvch[:, m, p : p + 1],
                        )
            if "ttr" not in SKIP:
                nc.vector.tensor_add(a["cvacc"][:], a["cvacc"][:], cvch[:])
            for m in range(3):
                ps_v = pp.tile([128, CH], F32, tag="ps")
                for k in range(3):
                    nc.tensor.matmul(ps_v[:], wts["qkv"][k][:, 1 + D + m * 128 : 1 + D + (m + 1) * 128].bitcast(F32R), zt[:, k * CH : (k + 1) * CH].bitcast(F32R), start=(k == 0), stop=(k == 2))
                vt = sp.tile([128, CH], F32, tag="vt", name="vt")
                nc.scalar.activation(out=vt[:], in_=ps_v[:], func=AF.Relu, bias=wts["vB"][:, m : m + 1])
                nc.sync.dma_start(out=vdst[m * 128 : (m + 1) * 128, chunk * CH : (chunk + 1) * CH], in_=vt[:])

        def finalize_cv(bi):
            a = att[bi]
            zsum = sp.tile([1, 4], F32, tag="zsum")
            zs = sp.tile([1, 4], F32, tag="zs")
            nc.vector.tensor_reduce(zsum[:], a["Zp"][:], axis=mybir.AxisListType.X, op=OP.add)
            nc.vector.reciprocal(zs[:], zsum[:])
            nc.sync.dma_start(out=zsd[bi : bi + 1, :], in_=zs[:])
            zb = sp.tile([128, 4], F32, tag="zb")
            zrow = zsd[bi, :]
            nc.sync.dma_start(out=zb[:], in_=bass.AP(tensor=zrow.tensor, offset=zrow.offset, ap=[[0, 128], [1, 4]]))
            for d in range(3):
                nc.vector.tensor_mul(a["cvf"][:, d, :], a["cvacc"][:, d, :], zb[:])

        def pass_b(bi, zt, vt, chunk, zdst, wp):
            """Pass B for block bi: returns new-z SBUF tile [128, 3*CH].

            zt: residual z chunk (f32), vt: relu'd v chunk (f32). Applies
            v*cv -> wo -> +z, then LN2/FFN residual. Writes result to zdst.
            """
            a = att[bi]
            wts = bw[bi]
            # v scaled by cv: per (dtile, patch) per-partition scalar (GPSIMD)
            for d in range(3):
                vv = patch_view(vt[:, d * CH : (d + 1) * CH])
                for ph in range(2):
                    for pw_ in range(2):
                        sub = vv[:, :, ph, :, pw_]
                        nc.gpsimd.tensor_scalar_mul(out=sub.bitcast(F32R), in0=sub, scalar1=a["cvf"][:, d, 2 * ph + pw_ : 2 * ph + pw_ + 1])
            # wo matmul + residual
            za = wp.tile([128, 3 * CH], F32, tag="za", name="za")
            for m in range(3):
                ps_o = pp.tile([128, CH], F32, tag="ps")
                for k in range(3):
                    nc.tensor.matmul(ps_o[:], wts["wo"][k][:, m * 128 : (m + 1) * 128].bitcast(F32R), vt[:, k * CH : (k + 1) * CH].bitcast(F32R), start=(k == 0), stop=(k == 2))
                nc.vector.scalar_tensor_tensor(out=za[:, m * CH : (m + 1) * CH].bitcast(F32R), in0=ps_o[:], scalar=wts["woB"][:, m : m + 1], in1=zt[:, m * CH : (m + 1) * CH], op0=OP.add, op1=OP.add)
            # LN2 stats on za
            ps_s = pp.tile([128, CH], F32, tag="ps")
            ps_q = pp.tile([128, CH], F32, tag="ps")
            for d in range(3):
                sq = sp.tile([128, CH], F32, tag="sq", name="sq")
                nc.scalar.activation(out=sq[:].bitcast(F32R), in_=za[:, d * CH : (d + 1) * CH], func=AF.Square)
                nc.tensor.matmul(ps_s[:], ones_t[:].bitcast(F32R), za[:, d * CH : (d + 1) * CH].bitcast(F32R), start=(d == 0), stop=(d == 2))
                nc.tensor.matmul(ps_q[:], ones_t[:].bitcast(F32R), sq[:].bitcast(F32R), start=(d == 0), stop=(d == 2))
            Mt = sp.tile([128, CH], F32, tag="Mt")
            Rt = sp.tile([128, CH], F32, tag="Rt")
            tmp = sp.tile([128, CH], F32, tag="tmp")
            nc.scalar.activation(out=Mt[:], in_=ps_s[:], func=AF.Copy, scale=1.0 / D)
            nc.vector.tensor_mul(tmp[:], Mt[:], Mt[:])
            nc.vector.scalar_tensor_tensor(out=tmp[:], in0=ps_q[:], scalar=1.0 / D, in1=tmp[:], op0=OP.mult, op1=OP.subtract)
            nc.scalar.activation(out=tmp[:], in_=tmp[:], func=AF.Sqrt, bias=eps_t[:])
            nc.vector.reciprocal(Rt[:], tmp[:])
            zn = wp.tile([128, 3 * CH], F32, tag="zn", name="zn")
            Mb = bass.AP(tensor=Mt[:].tensor, offset=Mt[:].offset, ap=[Mt[:].ap[0], [0, 3], [1, CH]])
            Rb = bass.AP(tensor=Rt[:].tensor, offset=Rt[:].offset, ap=[Rt[:].ap[0], [0, 3], [1, CH]])
            z3 = za[:].rearrange("p (d n) -> p d n", d=3)
            zn3 = zn[:].rearrange("p (d n) -> p d n", d=3)
            nc.vector.tensor_sub(zn3.bitcast(F32R), z3, Mb)
            nc.vector.tensor_mul(zn3.bitcast(F32R), zn3, Rb)
            # FFN: ff1 (6 m-tiles, silu) -> h, ff2 accumulate -> + za
            ps_f = [pp.tile([128, CH], F32, tag="ps", name=f"psf{m}") for m in range(3)]
            for third in range(3):
                ht = wp.tile([128, 2 * CH], F32, tag="ht", name="ht")
                for mi in range(2):
                    m = third * 2 + mi
                    ps_1 = pp.tile([128, CH], F32, tag="ps")
                    for k in range(3):
                        nc.tensor.matmul(ps_1[:], wts["ff1"][k][:, m * 128 : (m + 1) * 128].bitcast(F32R), zn[:, k * CH : (k + 1) * CH].bitcast(F32R), start=(k == 0), stop=(k == 2))
                    nc.scalar.activation(out=ht[:, mi * CH : (mi + 1) * CH].bitcast(F32R), in_=ps_1[:], func=(AF.Square if SIM_SAFE else AF.Silu), bias=wts["ff1B"][:, m : m + 1])
                for m in range(3):
                    for ki in range(2):
                        k = third * 2 + ki
                        nc.tensor.matmul(ps_f[m][:], wts["ff2"][k][:, m * 128 : (m + 1) * 128].bitcast(F32R), ht[:, ki * CH : (ki + 1) * CH].bitcast(F32R), start=(k == 0), stop=(k == 5))
            zb_ = sp.tile([128, 3 * CH], F32, tag="zt", name="zb_", bufs=4)
            for m in range(3):
                nc.vector.scalar_tensor_tensor(out=zb_[:, m * CH : (m + 1) * CH].bitcast(F32R), in0=ps_f[m][:], scalar=wts["ff2B"][:, m : m + 1], in1=za[:, m * CH : (m + 1) * CH], op0=OP.add, op1=OP.add)
                if zdst is not None:
                    nc.sync.dma_start(out=zdst[m * 128 : (m + 1) * 128, chunk * CH : (chunk + 1) * CH], in_=zb_[:, m * CH : (m + 1) * CH])
            return zb_

        # ================= PHASE 1: conv + pw1 + block0 pass A =================
        p1_cm = tc.tile_pool(name="p1", bufs=2)
        p1 = p1_cm.__enter__()
        xslices = {}

        def load_slice(ts_):
            xs = p1.tile([128, 2, PSL], F32, tag="xps", name="xps", bufs=4)
            nc.gpsimd.memset(xs[:], 0.0)
            for cti in range(2):
                dst = xs[:, cti, :].rearrange("p (h w) -> p h w", h=PW)
                nc.sync.dma_start(out=dst[:, 1:33, 1:33], in_=x_in[cti * 128 : (cti + 1) * 128, ts_, :, :])
            xslices[ts_] = xs

        for t in range(T):
            for ts_ in (t - 1, t, t + 1):
                if 0 <= ts_ < T and ts_ not in xslices:
                    load_slice(ts_)
            yact = p1.tile([128, 2, H * W], F32, tag="yact", name="yact")
            for cti in range(2):
                acc = p1.tile([128, H * W], F32, tag="acc", name="acc")
                accv = acc[:].rearrange("p (h w) -> p h w", h=H)
                first = True
                for dt in range(3):
                    ts_ = t + dt - 1
                    if not (0 <= ts_ < T):
                        continue
                    xv = xslices[ts_][:, cti, :].rearrange("p (h w) -> p h w", h=PW)
                    for dh in range(3):
                        for dw in range(3):
                            src = xv[:, dh : dh + H, dw : dw + W]
                            wcol = dw_t[:, cti, dt * 9 + dh * 3 + dw : dt * 9 + dh * 3 + dw + 1]
                            if first:
                                nc.vector.tensor_scalar_mul(out=accv, in0=src, scalar1=wcol)
                                first = False
                            else:
                                nc.vector.scalar_tensor_tensor(out=accv, in0=src, scalar=wcol, in1=accv, op0=OP.mult, op1=OP.add)
                nc.scalar.activation(out=yact[:, cti, :].bitcast(F32R), in_=acc[:], func=(AF.Square if SIM_SAFE else AF.Silu), bias=dwb_t[:, cti : cti + 1])
            for half in range(2):
                chunk = 2 * t + half
                zt = sp.tile([128, 3 * CH], F32, tag="zt", name="zt", bufs=4)
                for m in range(3):
                    ps1 = pp.tile([128, CH], F32, tag="ps")
                    for k in range(2):
                        nc.tensor.matmul(ps1[:], pw1_t[k][:, m * 128 : (m + 1) * 128].bitcast(F32R), yact[:, k, half * CH : (half + 1) * CH].bitcast(F32R), start=(k == 0), stop=(k == 1))
                    nc.scalar.activation(out=zt[:, m * CH : (m + 1) * CH].bitcast(F32R), in_=ps1[:], func=AF.Identity, bias=pw1b_t[:, m : m + 1])
                    nc.sync.dma_start(out=z0[m * 128 : (m + 1) * 128, chunk * CH : (chunk + 1) * CH], in_=zt[:, m * CH : (m + 1) * CH])
                if STAGE >= 2:
                    ln_then_qkv_attn(0, zt, chunk)
        p1_cm.__exit__(None, None, None)
        p23 = ctx.enter_context(tc.tile_pool(name="p23", bufs=2))
        NO_P2 = os.environ.get("KERNEL_NO_P2", "0") == "1"
        if STAGE >= 2:
            finalize_cv(0)

            # ============= PHASE 2: block0 pass B + block1 pass A =============
            for chunk in range(NCH if not NO_P2 else 0):
                zt = sp.tile([128, 3 * CH], F32, tag="zt", name="zt", bufs=4)
                vt = p23.tile([128, 3 * CH], F32, tag="vt2", name="vt2")
                for m in range(3):
                    nc.sync.dma_start(out=zt[:, m * CH : (m + 1) * CH], in_=z0[m * 128 : (m + 1) * 128, chunk * CH : (chunk + 1) * CH])
                    nc.sync.dma_start(out=vt[:, m * CH : (m + 1) * CH].bitcast(F32R), in_=v0[m * 128 : (m + 1) * 128, chunk * CH : (chunk + 1) * CH].bitcast(F32R))
                zb_ = pass_b(0, zt, vt, chunk, z1, p23)
                if STAGE >= 3:
                    ln_then_qkv_attn(1, zb_, chunk)
        if STAGE >= 3:
            finalize_cv(1)

            # ================= PHASE 3: block1 pass B + pw2 =================
            for chunk in range(NCH):
                zt = sp.tile([128, 3 * CH], F32, tag="zt", name="zt", bufs=4)
                vt = p23.tile([128, 3 * CH], F32, tag="vt2", name="vt2")
                for m in range(3):
                    nc.sync.dma_start(out=zt[:, m * CH : (m + 1) * CH], in_=z1[m * 128 : (m + 1) * 128, chunk * CH : (chunk + 1) * CH])
                    nc.sync.dma_start(out=vt[:, m * CH : (m + 1) * CH].bitcast(F32R), in_=v1[m * 128 : (m + 1) * 128, chunk * CH : (chunk + 1) * CH].bitcast(F32R))
                zb_ = pass_b(1, zt, vt, chunk, None, p23)
                ot = p23.tile([128, 2 * CH], F32, tag="ot", name="ot")
                for m in range(2):
                    ps2 = pp.tile([128, CH], F32, tag="ps")
                    for k in range(3):
                        nc.tensor.matmul(ps2[:], pw2_t[k][:, m * 128 : (m + 1) * 128].bitcast(F32R), zb_[:, k * CH : (k + 1) * CH].bitcast(F32R), start=(k == 0), stop=(k == 2))
                    nc.scalar.activation(out=ot[:, m * CH : (m + 1) * CH], in_=ps2[:], func=AF.Identity, bias=pw2b_t[:, m : m + 1])
                    nc.sync.dma_start(out=out[m * 128 : (m + 1) * 128, chunk * CH : (chunk + 1) * CH], in_=ot[:, m * CH : (m + 1) * CH])

    nc.compile()
    return nc


_NC = None


def _get_nc():
    global _NC
    if _NC is None:
        _NC = build()
    return _NC


def kernel(**inputs):
    inputs = {k: np.asarray(v, dtype=np.float32) for k, v in inputs.items()}
    x = inputs["x"]
    base = {
        "dwW": np.ascontiguousarray(inputs["dw_w"].reshape(C, 27)),
        "dwB": inputs["dw_b"].reshape(C, 1),
        "pw1W": np.ascontiguousarray(inputs["pw1_w"]),
        "pw1B": inputs["pw1_b"].reshape(D, 1),
        "pw2W": np.ascontiguousarray(inputs["pw2_w"]),
        "pw2B": inputs["pw2_b"].reshape(OUTC, 1),
    }
    for i in range(NBLK):
        qkvW = inputs["ln1_g"][i][:, None] * inputs["qkv_w"][i]
        qkvB = inputs["ln1_b"][i] @ inputs["qkv_w"][i] + inputs["qkv_b"][i]
        ff1W = inputs["ln2_g"][i][:, None] * inputs["ff1_w"][i]
        ff1B = inputs["ln2_b"][i] @ inputs["ff1_w"][i] + inputs["ff1_b"][i]
        base.update(
            {
                f"qkvW{i}": np.ascontiguousarray(qkvW),
                f"qB{i}": qkvB[0:1].reshape(1, 1),
                f"kB{i}": qkvB[1 : 1 + D].reshape(D, 1),
                f"vB{i}": qkvB[1 + D :].reshape(D, 1),
                f"woW{i}": np.ascontiguousarray(inputs["wo_w"][i]),
                f"woB{i}": inputs["wo_b"][i].reshape(D, 1),
                f"ff1W{i}": np.ascontiguousarray(ff1W),
                f"ff1B{i}": ff1B.reshape(FF, 1),
                f"ff2W{i}": np.ascontiguousarray(inputs["ff2_w"][i]),
                f"ff2B{i}": inputs["ff2_b"][i].reshape(D, 1),
            }
        )
    in_maps = [dict(base, x=np.ascontiguousarray(x[b])) for b in range(B)]
    nc = _get_nc()
    trace = bool(int(os.environ.get("KERNEL_TRACE", "0")))
    res = run_bass_kernel_spmd(nc, in_maps, list(range(B)), trace=trace)
    kernel.last_exec_ns = res.exec_time_ns
    kernel.last_profile = res.profile_json
    outs = [res.results[b]["out"].reshape(OUTC, T, H, W) for b in range(B)]
    kernel.last_results = res.results
    return np.stack(outs).astype(np.float32)

